# revision 93
# baseline (speedup 1.0000x reference)
"""Trainium2 Bass kernel for nn_BaseMultiHeadAttention (B=2, S=2048, E=1024, H=16).

Sharding: 4 heads x 1 batch per core (8 cores = 4 head-groups x 2 batches).
Each core runs RMSNorm + RoPE + causal attention for its 4 heads over the
full sequence of its batch, then a 256-feature row-shard of the output
projection; the host sums the 4 head-group partials per batch (the
all-reduce) and adds the bias.  Simulated HW time 135.8us/core (baseline
156.7us); measured rel err ~2e-3 (absmax) vs the 2e-2 gate.

Numerics: fp16 activations end-to-end (host converts; a consistent rope-pair
permutation keeps q.k dots unchanged), fp32 PSUM accumulation, bf16 output
projection, fp16 output partials.  RMSNorm bounds |scores*scale| <= 8 so exp
needs no max-subtraction and p = exp(s/8) in [1.1e-7, 3e3] fits fp16.

Engine assignment minds two constraints: GPSIMD cannot access PSUM (so all
PSUM evacuation lives on DVE), and each engine executes its stream in order
(so phase-B work must never queue behind long phase-A chains on the same
engine, and projection-tail stages are staged across exp-ring flushes so
score matmuls always sit between them in the PE stream).

Device pipeline per core:
  Phase A (per head, 4-tile waves issued just-in-time ~4 q-blocks ahead of
    use; the first waves run DVE-only to cut pipeline-fill latency): DMA
    fp16 q/k partition-major; sum-of-squares (Pool mul + DVE reduce);
    sqrt(mean+eps) on ACT; DVE reciprocal; RoPE split Pool/DVE (fp16 2x);
    DVE broadcast-normalize; PE-transpose fp16 s-tiles into a PSUM ring
    shared with the projection tail; DVE copies to qT/kT [64, S] fp16.
  Phase B (per q-block i, head-inner): scoresT[k,q] blocks [128,128] as
    fp16 matmuls stream into a ring of 2 PSUM tiles [128,1024]; ONE exp per
    full ring tile (ACT, scale=D^-0.5, fp16 out), packed across chunk/head
    boundaries to minimize instruction count on ACT, the steady-state
    pacer.  Causal masking of diagonal blocks costs zero extra engine ops:
    one extra PE matmul (lhsT=identity) accumulates a constant strict-
    lower-triangular -6e4 matrix onto the diagonal score block before exp.
    ctx[q,65] = p.T @ [v|1] accumulates per head in a shared [128,4,65]
    PSUM tile (ones column = softmax row sums); per-head DVE reciprocal +
    scale writes normalized bf16 context into a persistent [128,16,256]
    accumulator.  When all 4 heads of a q-block are done, the tail runs as
    deferred stages across successive ring flushes: 2 PE transposes ->
    DVE copy -> 4 bf16 matmuls (2-step 256-contraction) -> DVE PSUM
    evacuation -> fp16 DMA out; the last two q-blocks run immediately.
"""
import numpy as np

import bass_rust
import concourse.bass as bass
import concourse.mybir as mybir
import concourse.tile as tile
from concourse.bass_utils import run_bass_kernel_spmd
from concourse.masks import make_identity

B, S, E, H, D = 2, 2048, 1024, 16, 64
HD = D // 2
N_CORES = 8
NG = 4                     # head-groups (tensor parallel)
HL = H // NG               # 4 heads per core
NJ = HL                    # 4 jobs (heads) per core, single batch
NT = S // 128              # 16 s-tiles per head
EPS = 1.1920928955078125e-07
SCALE = float(D) ** -0.5
f32 = mybir.dt.float32
f16 = mybir.dt.float16
bf16 = mybir.dt.bfloat16
ALU = mybir.AluOpType
ACTF = mybir.ActivationFunctionType

_TC = tile.TileContext


def _legalize_waits(nc):
    """Split multi-wait sync_infos for this walrus build.

    This neuronxcc's codegen allows 1 sync wait per instruction (2 on
    EventSemaphore), while the Tile scheduler attaches all outstanding
    waits to one instruction.  Hoist the excess onto same-engine NoOps
    inserted immediately before the offending instruction — the engine
    executes its stream in order, so blocking semantics are identical.
    """
    uid = 0
    for f in nc.m.functions:
        for blk in f.blocks:
            insts = list(blk.instructions)
            out, changed = [], False
            for inst in insts:
                si = inst.sync_info
                cap = 2 if isinstance(inst, mybir.InstEventSemaphore) else 1
                if si is not None and len(si.on_wait) > cap:
                    changed = True
                    waits = list(si.on_wait)
                    for w in waits[:-cap]:
                        carrier = mybir.InstNoOp(
                            name=f"legwait-{uid}", engine=inst.engine,
                            ins=[], outs=[])
                        uid += 1
                        carrier.sync_info = bass_rust.SyncInfo(
                            on_wait=[w], on_update=[])
                        nc.register_instruction(carrier, overwrite=True)
                        out.append(carrier)
                    si.on_wait = waits[-cap:]
                    inst.sync_info = si
                out.append(inst)
            if changed:
                blk.instructions = out


def build_nc():
    nc = bass.Bass("TRN2", target_bir_lowering=False, debug=False)
    q_in = nc.dram_tensor("q", [NJ, 128, NT, D], f16, kind="ExternalInput")
    k_in = nc.dram_tensor("k", [NJ, 128, NT, D], f16, kind="ExternalInput")
    v_in = nc.dram_tensor("v", [NJ, 128, NT, D + 1], f16, kind="ExternalInput")
    cos_in = nc.dram_tensor("cos", [128, NT, HD], f16, kind="ExternalInput")
    sin_in = nc.dram_tensor("sin", [128, NT, HD], f16, kind="ExternalInput")
    wt_in = nc.dram_tensor("wt", [128, 2, E], bf16, kind="ExternalInput")
    out = nc.dram_tensor("out", [S, E], f16, kind="ExternalOutput")

    with _TC(nc) as tc:
        with tc.tile_pool(name="const", bufs=1) as cp, \
             tc.tile_pool(name="pa", bufs=3) as pa, \
             tc.tile_pool(name="pb", bufs=3) as pb, \
             tc.tile_pool(name="pp", bufs=6) as pp, \
             tc.tile_pool(name="po", bufs=10) as po, \
             tc.tile_pool(name="ps_s", bufs=2, space="PSUM") as ps_s, \
             tc.tile_pool(name="ps_c", bufs=2, space="PSUM") as ps_c, \
             tc.tile_pool(name="ps_o", bufs=2, space="PSUM") as ps_o:
            ident = cp.tile([128, 128], f16, name="ident")
            make_identity(nc, ident)
            identb = cp.tile([128, 128], bf16, name="identb")
            make_identity(nc, identb)
            ltmask = cp.tile([128, 128], f16, name="ltmask")
            nc.gpsimd.memset(ltmask, -60000.0)
            nc.gpsimd.affine_select(
                out=ltmask, in_=ltmask, compare_op=ALU.is_ge,
                fill=0.0, base=-1, pattern=[[-1, 128]],
                channel_multiplier=1)
            eps_t = cp.tile([128, 1], f32, name="eps_t")
            nc.vector.memset(eps_t, EPS)
            cos_sb = cp.tile([128, NT, HD], f16, name="cos_sb")
            sin_sb = cp.tile([128, NT, HD], f16, name="sin_sb")
            wt_sb = cp.tile([128, 2, E], bf16, name="wt_sb")
            qT = cp.tile([64, NJ, S], f16, name="qT")
            kT = cp.tile([64, NJ, S], f16, name="kT")
            vsb = cp.tile([128, NJ, NT, D + 1], f16, name="vsb")
            # normalized per-head context, all 16 q-blocks x 4 heads (bf16)
            cpall = cp.tile([128, NT, NJ * D], bf16, name="cpall")

            # ------------- Phase A: norm + rope + transpose -----------------
            def phase_a(j, sub, fast=False):
                ve = nc.vector if fast else nc.gpsimd
                t0, t1 = sub                    # s-tile range [t0, t1)
                ns = t1 - t0
                tsl = slice(t0, t1)
                raws = []
                ss = pa.tile([128, 2, ns], f32, tag=f"ss{ns}", name="ss")
                for idx, src in enumerate((q_in, k_in)):
                    raw = pa.tile([128, ns, D], f16, tag=f"raw{idx}{ns}",
                                  name="raw")
                    nc.sync.dma_start(out=raw, in_=src.ap()[j][:, tsl])
                    sq = pa.tile([128, ns, D], f16, tag=f"sq{ns}", name="sq")
                    ve.tensor_mul(sq, raw, raw)
                    nc.vector.reduce_sum(ss[:, idx, :], sq,
                                         axis=mybir.AxisListType.X)
                    raws.append(raw)
                rs = pa.tile([128, 2, ns], f32, tag=f"rs{ns}", name="rs")
                nc.scalar.activation(
                    out=rs, in_=ss, func=ACTF.Sqrt,
                    bias=eps_t, scale=1.0 / D,
                )
                nc.vector.reciprocal(out=rs, in_=rs)
                for idx, (raw, dstT) in enumerate(zip(raws, (qT, kT))):
                    x1, x2 = raw[:, :, 0:HD], raw[:, :, HD:D]
                    rn = pa.tile([128, ns, D], f16, tag=f"rn{ns}", name="rn")
                    t1_ = pa.tile([128, ns, HD], f16, tag=f"t1{ns}",
                                  name="t1_")
                    t2_ = pa.tile([128, ns, HD], f16, tag=f"t2{ns}",
                                  name="t2_")
                    csl, ssl = cos_sb[:, tsl, :], sin_sb[:, tsl, :]
                    ve.tensor_mul(t1_, x1, csl)
                    nc.vector.tensor_mul(t2_, x2, ssl)
                    nc.vector.tensor_sub(rn[:, :, 0:HD], t1_, t2_)
                    ve.tensor_mul(t1_, x1, ssl)
                    nc.vector.tensor_mul(t2_, x2, csl)
                    ve.tensor_add(rn[:, :, HD:D], t1_, t2_)
                    # normalize: rn * rs (broadcast over d via step-0 AP)
                    rsx = rs[:, idx, :]
                    rs_b = bass.AP(
                        tensor=rsx.tensor, offset=rsx.offset,
                        ap=[list(rsx.ap[0]), list(rsx.ap[1]), [0, D]])
                    xn = pa.tile([128, ns, D], f16, tag=f"xn{ns}", name="xn")
                    nc.vector.tensor_mul(xn, rn, rs_b)
                    for tg in range(ns // 4):
                        ps_tr = ps_o.tile([64, 512], f16, tag="o",
                                          name="ps_tr")
                        for tt in range(4):
                            t = tg * 4 + tt
                            nc.tensor.transpose(
                                ps_tr[:, tt * 128:(tt + 1) * 128],
                                xn[:, t, :], ident)
                        base = (t0 + tg * 4) * 128
                        nc.vector.tensor_copy(dstT[:, j, base:base + 512],
                                              ps_tr)

            def load_v(j):
                nc.sync.dma_start(out=vsb[:, j], in_=v_in.ap()[j])

            # ------------- Phase B: head-major attention --------------------
            # Head h's 136 score blocks stream i-ascending into a ring of 2
            # PSUM tiles [128,1024]; one exp per full tile; per-head ctx +
            # immediate normalize into cpall; projection tails fire during
            # the last head's pass, staged across flush boundaries.
            state = {
                "sps": None, "fill": 0, "pt": {}, "nhalf": 0,
                "pending": [], "blocks": {}, "actions": [],
            }
            ctx_done = [0] * NT

            def defer(delta, fn, prio=0):
                state["actions"].append(
                    (state["nhalf"] + delta, prio, fn))

            def run_due():
                due = [a for a in state["actions"] if a[0] <= state["nhalf"]]
                state["actions"] = [a for a in state["actions"]
                                    if a[0] > state["nhalf"]]
                for _, _, fn in sorted(due, key=lambda a: (a[0], a[1])):
                    fn()

            def flush_half():
                sps = state["sps"]
                g = state["nhalf"]
                pt = pp.tile([128, 1024], f16, tag="p", name="pt")
                nc.scalar.activation(out=pt, in_=sps, func=ACTF.Exp,
                                     scale=SCALE)
                state["pt"][g] = pt
                state["nhalf"] = g + 1
                state["sps"] = None
                state["fill"] = 0
                # Release chunk work one half LATE so the PE stream sees the
                # next half's score matmuls before ctx matmuls that wait on
                # this half's exp.
                rdy = [c for c in state["pending"]
                       if c[3] < state["nhalf"] - 1]
                state["pending"] = [c for c in state["pending"]
                                    if c[3] >= state["nhalf"] - 1]
                run_due()
                for (h, i, ctx, _) in rdy:
                    emit_ctx(h, i, ctx)

            def emit_block(h, i, jj):
                if state["sps"] is None:
                    state["sps"] = ps_s.tile([128, 1024], f32, tag="s",
                                             name="sps")
                off = state["fill"]
                diag = jj == i
                nc.tensor.matmul(
                    state["sps"][:, off * 128:(off + 1) * 128],
                    lhsT=kT[:, h, jj * 128:(jj + 1) * 128],
                    rhs=qT[:, h, i * 128:(i + 1) * 128],
                    start=True, stop=not diag,
                )
                if diag:
                    # causal mask: accumulate -6e4 onto the k > q half so
                    # exp() zeroes it -- no post-exp masking op needed
                    nc.tensor.matmul(
                        state["sps"][:, off * 128:(off + 1) * 128],
                        lhsT=ident, rhs=ltmask, start=False, stop=True,
                    )
                state["blocks"][(h, i, jj)] = (state["nhalf"], off)
                state["fill"] += 1
                if state["fill"] == 8:
                    flush_half()

            def emit_ctx(h, i, ctx):
                """ctx[q, 65] = sum_jj p(jj, i).T @ [v|1]; then normalize."""
                for jj in range(i + 1):
                    g, off = state["blocks"].pop((h, i, jj))
                    pt = state["pt"][g]
                    nc.tensor.matmul(
                        ctx[:, h, :],
                        lhsT=pt[:, off * 128:(off + 1) * 128],
                        rhs=vsb[:, h, jj, :],
                        start=(jj == 0), stop=(jj == i),
                    )
                rs1 = pb.tile([128, 1], f32, tag="rs1", name="rs1")
                nc.vector.reciprocal(out=rs1, in_=ctx[:, h, D:D + 1])
                nc.vector.tensor_scalar_mul(
                    cpall[:, i, h * D:(h + 1) * D], ctx[:, h, 0:D], rs1)
                ctx_done[i] += 1
                if ctx_done[i] == NJ:
                    if i >= NT - 2:
                        stage_transpose(i, immediate=True)
                    else:
                        defer(1, lambda i=i: stage_transpose(i), prio=1)

            def stage_transpose(i, immediate=False):
                trp = ps_o.tile([128, 256], bf16, tag="o", name="trp")
                for half in range(2):
                    nc.tensor.transpose(
                        trp[:, half * 128:(half + 1) * 128],
                        cpall[:, i, half * 128:(half + 1) * 128], identb)
                ctxT = pb.tile([128, 256], bf16, tag="ctxT", name="ctxT")
                nc.vector.tensor_copy(ctxT, trp)
                defer(1, lambda: stage_proj(i, ctxT), prio=0)

            def stage_proj(i, ctxT):
                last = i == NT - 1
                osb = po.tile([128, E], f16, tag="osb", name="osb")
                for n in range(2):
                    ops_ = ps_o.tile([128, 512], f32, tag="o", name="ops")
                    for half in range(2):
                        nc.tensor.matmul(
                            ops_, lhsT=ctxT[:, half * 128:(half + 1) * 128],
                            rhs=wt_sb[:, half, n * 512:(n + 1) * 512],
                            start=(half == 0), stop=(half == 1),
                        )
                    # final q-block: ACT is idle by now -- evacuate half on
                    # it and DMA each half separately to overlap the drain
                    if last and n == 0:
                        nc.scalar.copy(osb[:, n * 512:(n + 1) * 512], ops_)
                    else:
                        nc.vector.tensor_copy(
                            osb[:, n * 512:(n + 1) * 512], ops_)
                    if last:
                        nc.sync.dma_start(
                            out=out.ap()[i * 128:(i + 1) * 128,
                                         n * 512:(n + 1) * 512],
                            in_=osb[:, n * 512:(n + 1) * 512])
                if not last:
                    nc.sync.dma_start(
                        out=out.ap()[i * 128:(i + 1) * 128, :], in_=osb)

            nc.sync.dma_start(out=cos_sb, in_=cos_in.ap())
            nc.sync.dma_start(out=sin_sb, in_=sin_in.ap())
            for j in range(NJ):
                phase_a(j, (0, 4), fast=(j < 2))
            nc.sync.dma_start(out=wt_sb, in_=wt_in.ap())
            for j in range(NJ):
                load_v(j)
            for i in range(NT):
                if i < 12:
                    w = i // 4 + 1
                    j = i % 4
                    phase_a(j, (4 * w, 4 * w + 4), fast=(i < 3))
                ctx = ps_c.tile([128, NJ, D + 1], f32, tag="c", name="ctx")
                for h in range(NJ):
                    for jj in range(i + 1):
                        emit_block(h, i, jj)
                    if state["blocks"][(h, i, i)][0] < state["nhalf"] - 1:
                        emit_ctx(h, i, ctx)
                    else:
                        state["pending"].append(
                            (h, i, ctx, state["blocks"][(h, i, i)][0]))
            assert state["fill"] == 0, state["fill"]
            for (h, i, ctx, _) in state["pending"]:
                emit_ctx(h, i, ctx)
            state["pending"] = []
            while state["actions"]:
                state["nhalf"] += 1
                run_due()
    _legalize_waits(nc)
    return nc


# even rope lanes first, then odd — a consistent feature permutation of q/k
# leaves q.k dot products unchanged and makes every rope op a contiguous
# full-width DVE op
_ROPE_PERM = np.concatenate([np.arange(0, D, 2), np.arange(1, D, 2)])


def _shard_inputs(q, k, v, cos, sin, proj_w):
    """Per-core input maps (host-side layout/dtype prep only)."""
    qh = q.reshape(B, S, H, D)
    kh = k.reshape(B, S, H, D)
    vh = v.reshape(B, S, H, D)
    # [S, HD] -> [128, NT, HD] partition-major
    cos_t = np.ascontiguousarray(
        cos.reshape(NT, 128, HD).transpose(1, 0, 2)).astype(np.float16)
    sin_t = np.ascontiguousarray(
        sin.reshape(NT, 128, HD).transpose(1, 0, 2)).astype(np.float16)
    maps = []
    for core in range(N_CORES):
        g, b = core // B, core % B
        hs = slice(HL * g, HL * (g + 1))

        def tiles(x, permute, pad_ones=False):
            xs = x[b, :, hs, :].transpose(1, 0, 2)        # [HL, S, D]
            if permute:
                xs = xs[..., _ROPE_PERM]
            if pad_ones:
                ones = np.ones(xs.shape[:-1] + (1,), xs.dtype)
                xs = np.concatenate([xs, ones], axis=-1)
            d = xs.shape[-1]
            # [HL, NT, 128, d] -> [HL, 128, NT, d] partition-major
            return np.ascontiguousarray(
                xs.reshape(NJ, NT, 128, d).transpose(0, 2, 1, 3)
            ).astype(np.float16)

        # proj_w columns for this core's 256 features, as [128, 2, E]
        wt_c = np.ascontiguousarray(
            proj_w[:, 256 * g:256 * (g + 1)].T.reshape(2, 128, E)
            .transpose(1, 0, 2)).astype(bfloat16)
        maps.append({
            "q": tiles(qh, True), "k": tiles(kh, True),
            "v": tiles(vh, False, pad_ones=True),
            "cos": cos_t, "sin": sin_t, "wt": wt_c,
        })
    return maps


try:
    from ml_dtypes import bfloat16
except ImportError:  # pragma: no cover
    import jax.numpy as _jnp
    bfloat16 = _jnp.bfloat16

_NC_CACHE = []


def _get_nc():
    if not _NC_CACHE:
        _NC_CACHE.append(build_nc())
    return _NC_CACHE[0]


def kernel(q, k, v, attn_mask, padding_mask, qn_w, kn_w, proj_w, proj_b,
           cos, sin):
    q = np.asarray(q, np.float32)
    k = np.asarray(k, np.float32)
    v = np.asarray(v, np.float32)
    proj_w = np.asarray(proj_w, np.float32)
    proj_b = np.asarray(proj_b, np.float32)
    cos = np.asarray(cos, np.float32)
    sin = np.asarray(sin, np.float32)
    attn_mask = np.asarray(attn_mask)
    padding_mask = np.asarray(padding_mask)
    qn_w = np.asarray(qn_w, np.float32)
    kn_w = np.asarray(kn_w, np.float32)
    # The kernel bakes in: causal attn_mask, no padding, unit RMSNorm weights.
    assert np.array_equal(
        attn_mask.reshape(S, S), np.tril(np.ones((S, S), attn_mask.dtype)))
    assert padding_mask.all()
    assert np.all(qn_w == 1.0) and np.all(kn_w == 1.0)

    in_maps = _shard_inputs(q, k, v, cos, sin, proj_w)
    nc = _get_nc()
    res = run_bass_kernel_spmd(nc, in_maps, core_ids=list(range(N_CORES)))
    parts = np.stack([np.asarray(r["out"], np.float32)
                      for r in res.results])             # [8, S, E]
    full = np.empty((B, S, E), np.float32)
    for b in range(B):
        full[b] = parts[b::B].sum(axis=0) + proj_b[None, :]
    return full


# revision 94
# speedup vs baseline: 1.0078x; 1.0078x over previous
"""Trainium2 Bass kernel for nn_BaseMultiHeadAttention (B=2, S=2048, E=1024, H=16).

Sharding: 4 heads x 1 batch per core (8 cores = 4 head-groups x 2 batches).
Each core runs RMSNorm + RoPE + causal attention for its 4 heads over the
full sequence of its batch, then a 256-feature row-shard of the output
projection; the host sums the 4 head-group partials per batch (the
all-reduce) and adds the bias.  Simulated HW time 135.8us/core (baseline
156.7us); measured rel err ~2e-3 (absmax) vs the 2e-2 gate.

Numerics: fp16 activations end-to-end (host converts; a consistent rope-pair
permutation keeps q.k dots unchanged), fp32 PSUM accumulation, bf16 output
projection, fp16 output partials.  RMSNorm bounds |scores*scale| <= 8 so exp
needs no max-subtraction and p = exp(s/8) in [1.1e-7, 3e3] fits fp16.

Engine assignment minds two constraints: GPSIMD cannot access PSUM (so all
PSUM evacuation lives on DVE), and each engine executes its stream in order
(so phase-B work must never queue behind long phase-A chains on the same
engine, and projection-tail stages are staged across exp-ring flushes so
score matmuls always sit between them in the PE stream).

Device pipeline per core:
  Phase A (per head, 4-tile waves issued just-in-time ~4 q-blocks ahead of
    use; the first waves run DVE-only to cut pipeline-fill latency): DMA
    fp16 q/k partition-major; sum-of-squares (Pool mul + DVE reduce);
    sqrt(mean+eps) on ACT; DVE reciprocal; RoPE split Pool/DVE (fp16 2x);
    DVE broadcast-normalize; PE-transpose fp16 s-tiles into a PSUM ring
    shared with the projection tail; DVE copies to qT/kT [64, S] fp16.
  Phase B (per q-block i, head-inner): scoresT[k,q] blocks [128,128] as
    fp16 matmuls stream into a ring of 2 PSUM tiles [128,1024]; ONE exp per
    full ring tile (ACT, scale=D^-0.5, fp16 out), packed across chunk/head
    boundaries to minimize instruction count on ACT, the steady-state
    pacer.  Causal masking of diagonal blocks costs zero extra engine ops:
    one extra PE matmul (lhsT=identity) accumulates a constant strict-
    lower-triangular -6e4 matrix onto the diagonal score block before exp.
    ctx[q,65] = p.T @ [v|1] accumulates per head in a shared [128,4,65]
    PSUM tile (ones column = softmax row sums); per-head DVE reciprocal +
    scale writes normalized bf16 context into a persistent [128,16,256]
    accumulator.  When all 4 heads of a q-block are done, the tail runs as
    deferred stages across successive ring flushes: 2 PE transposes ->
    DVE copy -> 4 bf16 matmuls (2-step 256-contraction) -> DVE PSUM
    evacuation -> fp16 DMA out; the last two q-blocks run immediately.
"""
import numpy as np

import bass_rust
import concourse.bass as bass
import concourse.mybir as mybir
import concourse.tile as tile
from concourse.bass_utils import run_bass_kernel_spmd
from concourse.masks import make_identity

B, S, E, H, D = 2, 2048, 1024, 16, 64
HD = D // 2
N_CORES = 8
NG = 4                     # head-groups (tensor parallel)
HL = H // NG               # 4 heads per core
NJ = HL                    # 4 jobs (heads) per core, single batch
NT = S // 128              # 16 s-tiles per head
EPS = 1.1920928955078125e-07
SCALE = float(D) ** -0.5
f32 = mybir.dt.float32
f16 = mybir.dt.float16
bf16 = mybir.dt.bfloat16
ALU = mybir.AluOpType
ACTF = mybir.ActivationFunctionType

_TC = tile.TileContext


def _legalize_waits(nc):
    """Split multi-wait sync_infos for this walrus build.

    This neuronxcc's codegen allows 1 sync wait per instruction (2 on
    EventSemaphore), while the Tile scheduler attaches all outstanding
    waits to one instruction.  Hoist the excess onto same-engine NoOps
    inserted immediately before the offending instruction — the engine
    executes its stream in order, so blocking semantics are identical.
    """
    uid = 0
    for f in nc.m.functions:
        for blk in f.blocks:
            insts = list(blk.instructions)
            out, changed = [], False
            for inst in insts:
                si = inst.sync_info
                cap = 2 if isinstance(inst, mybir.InstEventSemaphore) else 1
                if si is not None and len(si.on_wait) > cap:
                    changed = True
                    waits = list(si.on_wait)
                    for w in waits[:-cap]:
                        carrier = mybir.InstNoOp(
                            name=f"legwait-{uid}", engine=inst.engine,
                            ins=[], outs=[])
                        uid += 1
                        carrier.sync_info = bass_rust.SyncInfo(
                            on_wait=[w], on_update=[])
                        nc.register_instruction(carrier, overwrite=True)
                        out.append(carrier)
                    si.on_wait = waits[-cap:]
                    inst.sync_info = si
                out.append(inst)
            if changed:
                blk.instructions = out


def build_nc():
    nc = bass.Bass("TRN2", target_bir_lowering=False, debug=False)
    q_in = nc.dram_tensor("q", [NJ, 128, NT, D], f16, kind="ExternalInput")
    k_in = nc.dram_tensor("k", [NJ, 128, NT, D], f16, kind="ExternalInput")
    v_in = nc.dram_tensor("v", [NJ, 128, NT, D + 1], f16, kind="ExternalInput")
    cos_in = nc.dram_tensor("cos", [128, NT, HD], f16, kind="ExternalInput")
    sin_in = nc.dram_tensor("sin", [128, NT, HD], f16, kind="ExternalInput")
    wt_in = nc.dram_tensor("wt", [128, 2, E], bf16, kind="ExternalInput")
    out = nc.dram_tensor("out", [S, E], f16, kind="ExternalOutput")

    with _TC(nc) as tc:
        with tc.tile_pool(name="const", bufs=1) as cp, \
             tc.tile_pool(name="pa", bufs=3) as pa, \
             tc.tile_pool(name="pb", bufs=3) as pb, \
             tc.tile_pool(name="pp", bufs=6) as pp, \
             tc.tile_pool(name="po", bufs=10) as po, \
             tc.tile_pool(name="ps_s", bufs=2, space="PSUM") as ps_s, \
             tc.tile_pool(name="ps_c", bufs=2, space="PSUM") as ps_c, \
             tc.tile_pool(name="ps_o", bufs=2, space="PSUM") as ps_o:
            ident = cp.tile([128, 128], f16, name="ident")
            make_identity(nc, ident)
            identb = cp.tile([128, 128], bf16, name="identb")
            make_identity(nc, identb)
            ltmask = cp.tile([128, 128], f16, name="ltmask")
            nc.gpsimd.memset(ltmask, -60000.0)
            nc.gpsimd.affine_select(
                out=ltmask, in_=ltmask, compare_op=ALU.is_ge,
                fill=0.0, base=-1, pattern=[[-1, 128]],
                channel_multiplier=1)
            eps_t = cp.tile([128, 1], f32, name="eps_t")
            nc.vector.memset(eps_t, EPS)
            cos_sb = cp.tile([128, NT, HD], f16, name="cos_sb")
            sin_sb = cp.tile([128, NT, HD], f16, name="sin_sb")
            wt_sb = cp.tile([128, 2, E], bf16, name="wt_sb")
            qT = cp.tile([64, NJ, S], f16, name="qT")
            kT = cp.tile([64, NJ, S], f16, name="kT")
            vsb = cp.tile([128, NJ, NT, D + 1], f16, name="vsb")
            # normalized per-head context, all 16 q-blocks x 4 heads (bf16)
            cpall = cp.tile([128, NT, NJ * D], bf16, name="cpall")

            # ------------- Phase A: norm + rope + transpose -----------------
            def phase_a(j, sub, fast=False):
                ve = nc.vector if fast else nc.gpsimd
                t0, t1 = sub                    # s-tile range [t0, t1)
                ns = t1 - t0
                tsl = slice(t0, t1)
                raws = []
                ss = pa.tile([128, 2, ns], f32, tag=f"ss{ns}", name="ss")
                for idx, src in enumerate((q_in, k_in)):
                    raw = pa.tile([128, ns, D], f16, tag=f"raw{idx}{ns}",
                                  name="raw")
                    nc.sync.dma_start(out=raw, in_=src.ap()[j][:, tsl])
                    sq = pa.tile([128, ns, D], f16, tag=f"sq{ns}", name="sq")
                    ve.tensor_mul(sq, raw, raw)
                    nc.vector.reduce_sum(ss[:, idx, :], sq,
                                         axis=mybir.AxisListType.X)
                    raws.append(raw)
                rs = pa.tile([128, 2, ns], f32, tag=f"rs{ns}", name="rs")
                nc.scalar.activation(
                    out=rs, in_=ss, func=ACTF.Sqrt,
                    bias=eps_t, scale=1.0 / D,
                )
                nc.vector.reciprocal(out=rs, in_=rs)
                for idx, (raw, dstT) in enumerate(zip(raws, (qT, kT))):
                    x1, x2 = raw[:, :, 0:HD], raw[:, :, HD:D]
                    rn = pa.tile([128, ns, D], f16, tag=f"rn{ns}", name="rn")
                    t1_ = pa.tile([128, ns, HD], f16, tag=f"t1{ns}",
                                  name="t1_")
                    t2_ = pa.tile([128, ns, HD], f16, tag=f"t2{ns}",
                                  name="t2_")
                    csl, ssl = cos_sb[:, tsl, :], sin_sb[:, tsl, :]
                    ve.tensor_mul(t1_, x1, csl)
                    nc.vector.tensor_mul(t2_, x2, ssl)
                    nc.vector.tensor_sub(rn[:, :, 0:HD], t1_, t2_)
                    ve.tensor_mul(t1_, x1, ssl)
                    nc.vector.tensor_mul(t2_, x2, csl)
                    ve.tensor_add(rn[:, :, HD:D], t1_, t2_)
                    # normalize: rn * rs (broadcast over d via step-0 AP)
                    rsx = rs[:, idx, :]
                    rs_b = bass.AP(
                        tensor=rsx.tensor, offset=rsx.offset,
                        ap=[list(rsx.ap[0]), list(rsx.ap[1]), [0, D]])
                    xn = pa.tile([128, ns, D], f16, tag=f"xn{ns}", name="xn")
                    nc.vector.tensor_mul(xn, rn, rs_b)
                    for tg in range(ns // 4):
                        ps_tr = ps_o.tile([64, 512], f16, tag="o",
                                          name="ps_tr")
                        for tt in range(4):
                            t = tg * 4 + tt
                            nc.tensor.transpose(
                                ps_tr[:, tt * 128:(tt + 1) * 128],
                                xn[:, t, :], ident)
                        base = (t0 + tg * 4) * 128
                        nc.vector.tensor_copy(dstT[:, j, base:base + 512],
                                              ps_tr)

            def load_v(j):
                nc.sync.dma_start(out=vsb[:, j], in_=v_in.ap()[j])

            # ------------- Phase B: head-major attention --------------------
            # Head h's 136 score blocks stream i-ascending into a ring of 2
            # PSUM tiles [128,1024]; one exp per full tile; per-head ctx +
            # immediate normalize into cpall; projection tails fire during
            # the last head's pass, staged across flush boundaries.
            state = {
                "sps": None, "fill": 0, "pt": {}, "nhalf": 0,
                "pending": [], "blocks": {}, "actions": [],
            }
            ctx_done = [0] * NT

            def defer(delta, fn, prio=0):
                state["actions"].append(
                    (state["nhalf"] + delta, prio, fn))

            def run_due():
                due = [a for a in state["actions"] if a[0] <= state["nhalf"]]
                state["actions"] = [a for a in state["actions"]
                                    if a[0] > state["nhalf"]]
                for _, _, fn in sorted(due, key=lambda a: (a[0], a[1])):
                    fn()

            def flush_half():
                sps = state["sps"]
                g = state["nhalf"]
                pt = pp.tile([128, 1024], f16, tag="p", name="pt")
                nc.scalar.activation(out=pt, in_=sps, func=ACTF.Exp,
                                     scale=SCALE)
                state["pt"][g] = pt
                state["nhalf"] = g + 1
                state["sps"] = None
                state["fill"] = 0
                # Release chunk work one half LATE so the PE stream sees the
                # next half's score matmuls before ctx matmuls that wait on
                # this half's exp.
                rdy = [c for c in state["pending"]
                       if c[3] < state["nhalf"] - 1]
                state["pending"] = [c for c in state["pending"]
                                    if c[3] >= state["nhalf"] - 1]
                run_due()
                for (h, i, ctx, _) in rdy:
                    emit_ctx(h, i, ctx)

            def emit_block(h, i, jj):
                if state["sps"] is None:
                    state["sps"] = ps_s.tile([128, 1024], f32, tag="s",
                                             name="sps")
                off = state["fill"]
                diag = jj == i
                nc.tensor.matmul(
                    state["sps"][:, off * 128:(off + 1) * 128],
                    lhsT=kT[:, h, jj * 128:(jj + 1) * 128],
                    rhs=qT[:, h, i * 128:(i + 1) * 128],
                    start=True, stop=not diag,
                )
                if diag:
                    # causal mask: accumulate -6e4 onto the k > q half so
                    # exp() zeroes it -- no post-exp masking op needed
                    nc.tensor.matmul(
                        state["sps"][:, off * 128:(off + 1) * 128],
                        lhsT=ident, rhs=ltmask, start=False, stop=True,
                    )
                state["blocks"][(h, i, jj)] = (state["nhalf"], off)
                state["fill"] += 1
                if state["fill"] == 8:
                    flush_half()

            def emit_ctx(h, i, ctx):
                """ctx[q, 65] = sum_jj p(jj, i).T @ [v|1]; then normalize."""
                for jj in range(i + 1):
                    g, off = state["blocks"].pop((h, i, jj))
                    pt = state["pt"][g]
                    nc.tensor.matmul(
                        ctx[:, h, :],
                        lhsT=pt[:, off * 128:(off + 1) * 128],
                        rhs=vsb[:, h, jj, :],
                        start=(jj == 0), stop=(jj == i),
                    )
                rs1 = pb.tile([128, 1], f32, tag="rs1", name="rs1")
                nc.vector.reciprocal(out=rs1, in_=ctx[:, h, D:D + 1])
                nc.vector.tensor_scalar_mul(
                    cpall[:, i, h * D:(h + 1) * D], ctx[:, h, 0:D], rs1)
                ctx_done[i] += 1
                if ctx_done[i] == NJ:
                    if i >= NT - 2:
                        stage_transpose(i, immediate=True)
                    else:
                        defer(1, lambda i=i: stage_transpose(i), prio=1)

            def stage_transpose(i, immediate=False):
                trp = ps_o.tile([128, 256], bf16, tag="o", name="trp")
                for half in range(2):
                    nc.tensor.transpose(
                        trp[:, half * 128:(half + 1) * 128],
                        cpall[:, i, half * 128:(half + 1) * 128], identb)
                ctxT = pb.tile([128, 256], bf16, tag="ctxT", name="ctxT")
                nc.vector.tensor_copy(ctxT, trp)
                defer(1, lambda: stage_proj(i, ctxT), prio=0)

            def stage_proj(i, ctxT):
                last = i == NT - 1
                osb = po.tile([128, E], f16, tag="osb", name="osb")
                for n in range(2):
                    ops_ = ps_o.tile([128, 512], f32, tag="o", name="ops")
                    for half in range(2):
                        nc.tensor.matmul(
                            ops_, lhsT=ctxT[:, half * 128:(half + 1) * 128],
                            rhs=wt_sb[:, half, n * 512:(n + 1) * 512],
                            start=(half == 0), stop=(half == 1),
                        )
                    # final q-block: ACT is idle by now -- evacuate half on
                    # it and DMA each half separately to overlap the drain
                    if last and n == 0:
                        nc.scalar.copy(osb[:, n * 512:(n + 1) * 512], ops_)
                    else:
                        nc.vector.tensor_copy(
                            osb[:, n * 512:(n + 1) * 512], ops_)
                    if last:
                        nc.sync.dma_start(
                            out=out.ap()[i * 128:(i + 1) * 128,
                                         n * 512:(n + 1) * 512],
                            in_=osb[:, n * 512:(n + 1) * 512])
                if not last:
                    nc.sync.dma_start(
                        out=out.ap()[i * 128:(i + 1) * 128, :], in_=osb)

            nc.sync.dma_start(out=cos_sb, in_=cos_in.ap())
            nc.sync.dma_start(out=sin_sb, in_=sin_in.ap())
            for j in range(NJ):
                phase_a(j, (0, 4), fast=(j < 2))
            nc.sync.dma_start(out=wt_sb, in_=wt_in.ap())
            for j in range(NJ):
                load_v(j)
            for i in range(NT):
                if i < 12:
                    w = i // 4 + 1
                    j = i % 4
                    phase_a(j, (4 * w, 4 * w + 4), fast=(i < 2))
                ctx = ps_c.tile([128, NJ, D + 1], f32, tag="c", name="ctx")
                for h in range(NJ):
                    for jj in range(i + 1):
                        emit_block(h, i, jj)
                    if state["blocks"][(h, i, i)][0] < state["nhalf"] - 1:
                        emit_ctx(h, i, ctx)
                    else:
                        state["pending"].append(
                            (h, i, ctx, state["blocks"][(h, i, i)][0]))
            assert state["fill"] == 0, state["fill"]
            for (h, i, ctx, _) in state["pending"]:
                emit_ctx(h, i, ctx)
            state["pending"] = []
            while state["actions"]:
                state["nhalf"] += 1
                run_due()
    _legalize_waits(nc)
    return nc


# even rope lanes first, then odd — a consistent feature permutation of q/k
# leaves q.k dot products unchanged and makes every rope op a contiguous
# full-width DVE op
_ROPE_PERM = np.concatenate([np.arange(0, D, 2), np.arange(1, D, 2)])


def _shard_inputs(q, k, v, cos, sin, proj_w):
    """Per-core input maps (host-side layout/dtype prep only)."""
    qh = q.reshape(B, S, H, D)
    kh = k.reshape(B, S, H, D)
    vh = v.reshape(B, S, H, D)
    # [S, HD] -> [128, NT, HD] partition-major
    cos_t = np.ascontiguousarray(
        cos.reshape(NT, 128, HD).transpose(1, 0, 2)).astype(np.float16)
    sin_t = np.ascontiguousarray(
        sin.reshape(NT, 128, HD).transpose(1, 0, 2)).astype(np.float16)
    maps = []
    for core in range(N_CORES):
        g, b = core // B, core % B
        hs = slice(HL * g, HL * (g + 1))

        def tiles(x, permute, pad_ones=False):
            xs = x[b, :, hs, :].transpose(1, 0, 2)        # [HL, S, D]
            if permute:
                xs = xs[..., _ROPE_PERM]
            if pad_ones:
                ones = np.ones(xs.shape[:-1] + (1,), xs.dtype)
                xs = np.concatenate([xs, ones], axis=-1)
            d = xs.shape[-1]
            # [HL, NT, 128, d] -> [HL, 128, NT, d] partition-major
            return np.ascontiguousarray(
                xs.reshape(NJ, NT, 128, d).transpose(0, 2, 1, 3)
            ).astype(np.float16)

        # proj_w columns for this core's 256 features, as [128, 2, E]
        wt_c = np.ascontiguousarray(
            proj_w[:, 256 * g:256 * (g + 1)].T.reshape(2, 128, E)
            .transpose(1, 0, 2)).astype(bfloat16)
        maps.append({
            "q": tiles(qh, True), "k": tiles(kh, True),
            "v": tiles(vh, False, pad_ones=True),
            "cos": cos_t, "sin": sin_t, "wt": wt_c,
        })
    return maps


try:
    from ml_dtypes import bfloat16
except ImportError:  # pragma: no cover
    import jax.numpy as _jnp
    bfloat16 = _jnp.bfloat16

_NC_CACHE = []


def _get_nc():
    if not _NC_CACHE:
        _NC_CACHE.append(build_nc())
    return _NC_CACHE[0]


def kernel(q, k, v, attn_mask, padding_mask, qn_w, kn_w, proj_w, proj_b,
           cos, sin):
    q = np.asarray(q, np.float32)
    k = np.asarray(k, np.float32)
    v = np.asarray(v, np.float32)
    proj_w = np.asarray(proj_w, np.float32)
    proj_b = np.asarray(proj_b, np.float32)
    cos = np.asarray(cos, np.float32)
    sin = np.asarray(sin, np.float32)
    attn_mask = np.asarray(attn_mask)
    padding_mask = np.asarray(padding_mask)
    qn_w = np.asarray(qn_w, np.float32)
    kn_w = np.asarray(kn_w, np.float32)
    # The kernel bakes in: causal attn_mask, no padding, unit RMSNorm weights.
    assert np.array_equal(
        attn_mask.reshape(S, S), np.tril(np.ones((S, S), attn_mask.dtype)))
    assert padding_mask.all()
    assert np.all(qn_w == 1.0) and np.all(kn_w == 1.0)

    in_maps = _shard_inputs(q, k, v, cos, sin, proj_w)
    nc = _get_nc()
    res = run_bass_kernel_spmd(nc, in_maps, core_ids=list(range(N_CORES)))
    parts = np.stack([np.asarray(r["out"], np.float32)
                      for r in res.results])             # [8, S, E]
    full = np.empty((B, S, E), np.float32)
    for b in range(B):
        full[b] = parts[b::B].sum(axis=0) + proj_b[None, :]
    return full
